# revision 1
# baseline (speedup 1.0000x reference)
"""Fused BN(inference)+ReLU -> 1x1 conv (512->256) -> 2x2 avgpool on 8 TRN2 cores.

Full inputs in, full output out. Data-parallel over batch (16 -> 2 per core),
BN params + conv weights replicated.

Math folding (host side, tiny):
  s = bn_weight / sqrt(bn_var + eps)            [512]
  t = bn_bias - bn_mean * s                     [512]
  y = relu(s * x + t)                           (one ACT op per channel tile)
  avgpool2x2(W @ y) == (0.25 * W) @ sumpool2x2(y)   (pool before matmul: 4x
                                                     fewer matmul FLOPs)
  wt = 0.25 * W.T                               [512, 256] (lhsT layout)
"""

import copy as _copy

import numpy as np

import bass_rust
import concourse.bass as bass
import concourse.mybir as mybir
import concourse.tile as tile_mod
from concourse.bass_utils import run_bass_kernel_spmd

EPS = 1e-5

B, C_IN, C_OUT, H, W = 16, 512, 256, 56, 56
N_CORES = 8
B_PC = B // N_CORES          # batches per core
HW = H * W                   # 3136
HWP = (H // 2) * (W // 2)    # 784 pooled spatial
K_TILES = C_IN // 128        # 4
M_TILES = C_OUT // 128       # 2
N_CHUNK = HWP // 2           # 392 (fits one PSUM bank)

_DT = mybir.dt.float32


# This walrus build enforces per-instruction sync-wait caps that Tile's
# add_semaphores pass does not respect: CTRL-type instructions (Drain, NoOp)
# take no sem-ge waits at all, EventSemaphore takes at most 2, and every
# other instruction takes at most 1. Post-pass: hoist excess waits onto
# EventSemaphore carrier instructions inserted just before the owning
# instruction on the same engine (same blocking semantics - the carrier
# blocks the engine's sequencer until its waits pass).
_CTRL_OPS = ("InstDrain", "InstNoOp")


def _hoist_excess_waits(nc):
    ev_counter = [0]

    def make_carrier(engine, waits):
        ev_counter[0] += 1
        return mybir.InstEventSemaphore(
            name=f"EVHOIST-{ev_counter[0]}",
            engine=engine,
            ins=[],
            outs=[],
            sync_info=bass_rust.SyncInfo(on_wait=waits, on_update=[]),
        )

    new_module = _copy.replace(nc.m, functions=[])
    for function in nc.m.functions:
        new_function = _copy.replace(function, blocks=[])
        new_function.set_allocations_from_list(function.allocations)
        for block in function.blocks:
            new_insts = []
            for ins in block.instructions:
                si = ins.sync_info
                waits = list(si.on_wait) if si is not None else []
                opname = type(ins).__name__
                if opname in _CTRL_OPS:
                    keep = [w for w in waits if w.wait_mode != "sem-ge-imm"]
                    excess = [w for w in waits if w.wait_mode == "sem-ge-imm"]
                else:
                    limit = 2 if opname == "InstEventSemaphore" else 1
                    keep, excess = waits[:limit], waits[limit:]
                if excess:
                    for i in range(0, len(excess), 2):
                        new_insts.append(make_carrier(ins.engine, excess[i : i + 2]))
                    si.on_wait = keep
                new_insts.append(ins)
            new_function.blocks.append(_copy.replace(block, instructions=new_insts))
        new_module.functions.append(new_function)
    nc.m = new_module


def build_bass():
    nc = bass.Bass()

    # Params come pre-transposed from the host into partition-major layouts so
    # their DMAs are fully contiguous (the naive "(k p) -> p k" gather is 512
    # tiny reads and stalls the HWDGE FIFO ahead of the x stream).
    x_d = nc.dram_tensor("x", [B_PC, C_IN, H, W], _DT, kind="ExternalInput")
    s_d = nc.dram_tensor("s", [128, K_TILES], _DT, kind="ExternalInput")
    t_d = nc.dram_tensor("t", [128, K_TILES], _DT, kind="ExternalInput")
    wt_d = nc.dram_tensor(
        "wt", [128, K_TILES, C_OUT], _DT, kind="ExternalInput"
    )
    out_d = nc.dram_tensor(
        "out", [B_PC, C_OUT, H // 2, W // 2], _DT, kind="ExternalOutput"
    )

    with tile_mod.TileContext(nc) as tc:
        with (
            tc.tile_pool(name="const", bufs=1) as cpool,
            tc.tile_pool(name="xs", bufs=6) as xpool,
            tc.tile_pool(name="ys", bufs=5) as ypool,
            tc.tile_pool(name="us", bufs=4) as upool,
            tc.tile_pool(name="ps", bufs=3) as ppool,
            tc.tile_pool(name="os", bufs=6) as opool,
            tc.tile_pool(name="psum", bufs=8, space="PSUM") as pspool,
        ):
            # Replicated params, contiguous partition-major DMAs. They go on
            # the SAME sync HWDGE FIFO as the x stream, ahead of it: the
            # other HWDGE queue gets starved to ~45 GB/s once the x stream
            # saturates HBM, which held the first RELU hostage for ~14 us.
            # First x half-chunk goes FIRST on the FIFO: its consumer chain
            # is longest; params land ~1.5us later and are needed later.
            x00 = xpool.tile([128, 28 * W], _DT, tag="x", name="x_0_0_0")
            nc.sync.dma_start(
                out=x00[:],
                in_=x_d[0, 0:128, 0:28].rearrange("ch h w -> ch (h w)"),
            )
            s_sb = cpool.tile([128, K_TILES], _DT)
            nc.sync.dma_start(out=s_sb[:], in_=s_d[:])
            t_sb = cpool.tile([128, K_TILES], _DT)
            nc.sync.dma_start(out=t_sb[:], in_=t_d[:])
            wt_sb = cpool.tile([128, K_TILES, C_OUT], _DT)
            nc.sync.dma_start(out=wt_sb[:], in_=wt_d[:])
            # Trigger the lazy ACT Relu table load now, off the critical path
            warm = cpool.tile([1, 1], _DT)
            nc.scalar.activation(
                warm[:], s_sb[0:1, 0:1], mybir.ActivationFunctionType.Relu
            )

            def emit_chunk(b, k, row0, nrows, psums, first_k, last_k,
                           x_pre=None):
                """Process input rows [row0, row0+nrows) of k-slice k:
                DMA -> BN+ReLU -> 2x2 sum-pool -> matmul into psum pieces.

                nrows must be a multiple of 14 (half an n-chunk of pooled
                columns). PSUM first-write semantics: the matmul covering an
                n-chunk's column 0 at k==0 carries start=True (whole-bank
                has_written clear); later same-k pieces write with
                start=False and land as overwrites on the cleared bits.
                """
                c = row0 // 14
                hc = nrows * W
                if x_pre is not None:
                    x_t = x_pre
                else:
                    x_t = xpool.tile(
                        [128, hc], _DT, tag="x", name=f"x_{b}_{k}_{c}"
                    )
                    nc.sync.dma_start(
                        out=x_t[:],
                        in_=x_d[
                            b,
                            k * 128 : (k + 1) * 128,
                            row0 : row0 + nrows,
                        ].rearrange("ch h w -> ch (h w)"),
                    )
                y_t = ypool.tile([128, hc], _DT, tag="y", name=f"y_{b}_{k}_{c}")
                nc.scalar.activation(
                    y_t[:],
                    x_t[:],
                    mybir.ActivationFunctionType.Relu,
                    bias=t_sb[:, k : k + 1],
                    scale=s_sb[:, k : k + 1],
                )
                # H-pairs first: operands are contiguous 56-elem runs
                # (W-pairs first would be stride-2 reads on the big add)
                u_t = upool.tile(
                    [128, hc // 2], _DT, tag="u", name=f"u_{b}_{k}_{c}"
                )
                yv = y_t[:].rearrange("p (h two w) -> p h two w", two=2, w=W)
                nc.vector.tensor_add(u_t[:], yv[:, :, 0, :], yv[:, :, 1, :])
                # then W-pairs
                p_t = ppool.tile(
                    [128, hc // 4], _DT, tag="p", name=f"p_{b}_{k}_{c}"
                )
                uv = u_t[:].rearrange("p (a two) -> p a two", two=2)
                nc.vector.tensor_add(p_t[:], uv[:, :, 0], uv[:, :, 1])
                # map this chunk's pooled columns onto psum n-chunk pieces
                pooled0 = (row0 // 2) * (W // 2)  # global pooled col offset
                pooled_w = (nrows // 2) * (W // 2)
                for m in range(M_TILES):
                    off = 0
                    while off < pooled_w:
                        g = pooled0 + off  # global pooled col
                        n = g // N_CHUNK
                        col = g % N_CHUNK
                        width = min(N_CHUNK - col, pooled_w - off)
                        if first_k and (m, n) not in psums:
                            psums[(m, n)] = pspool.tile(
                                [128, N_CHUNK],
                                _DT,
                                tag="psum",
                                name=f"psum_{b}_{m}_{n}",
                            )
                        nc.tensor.matmul(
                            psums[(m, n)][:, col : col + width],
                            wt_sb[:, k, m * 128 : (m + 1) * 128],
                            p_t[:, off : off + width],
                            start=(first_k and col == 0),
                            stop=(last_k and col + width == N_CHUNK),
                            skip_group_check=True,
                        )
                        off += width

            for b in range(B_PC):
                psums = {}
                for k in range(K_TILES):
                    first_k = k == 0
                    last_k = k == K_TILES - 1
                    edge_first = b == 0 and k == 0
                    edge_last = b == B_PC - 1 and k == K_TILES - 1
                    if edge_first or edge_last:
                        # half chunks at the global pipeline edges
                        for q in range(2):
                            emit_chunk(
                                b, k, q * 28, 28, psums, first_k, last_k,
                                x_pre=x00 if edge_first and q == 0 else None,
                            )
                    else:
                        emit_chunk(b, k, 0, H, psums, first_k, last_k)

                out_v = out_d[:].rearrange("bb o h w -> bb o (h w)")
                for m in range(M_TILES):
                    for n in range(2):
                        # PSUM -> SBUF (DMA can't read PSUM); alternate
                        # engines, ship each half as soon as it's staged
                        o_t = opool.tile(
                            [128, N_CHUNK], _DT, tag="o", name=f"o_{b}_{m}_{n}"
                        )
                        if n == 0:
                            nc.scalar.copy(o_t[:], psums[(m, n)][:])
                        else:
                            nc.vector.tensor_copy(o_t[:], psums[(m, n)][:])
                        out_eng = nc.sync if n == 0 else nc.scalar
                        out_eng.dma_start(
                            out=out_v[
                                b,
                                m * 128 : (m + 1) * 128,
                                n * N_CHUNK : (n + 1) * N_CHUNK,
                            ],
                            in_=o_t[:],
                        )
    _hoist_excess_waits(nc)
    return nc


_NC_CACHE = None


def _get_nc():
    global _NC_CACHE
    if _NC_CACHE is None:
        _NC_CACHE = build_bass()
    return _NC_CACHE


def _prep_host(bn_weight, bn_bias, bn_mean, bn_var, conv_weight):
    s = (bn_weight / np.sqrt(bn_var + EPS)).astype(np.float32)
    t = (bn_bias - bn_mean * s).astype(np.float32)
    wt = (0.25 * conv_weight.T).astype(np.float32)  # [C_IN, C_OUT]
    # partition-major layouts: [128, K] for vectors, [128, K, C_OUT] for wt
    s2 = np.ascontiguousarray(s.reshape(K_TILES, 128).T)
    t2 = np.ascontiguousarray(t.reshape(K_TILES, 128).T)
    wt2 = np.ascontiguousarray(
        wt.reshape(K_TILES, 128, C_OUT).transpose(1, 0, 2)
    )
    return s2, t2, wt2


def _install_ntff_hook():
    # The agent image's antenv lacks axon_hooks; synthesize it from the boot
    # shim's ctypes factory so trace=True captures NTFF profiles.
    import sys
    import types

    try:
        import antenv.axon_hooks  # noqa: F401

        return
    except ImportError:
        pass
    from trn_agent_boot.trn_boot import _ntff_profile_via_ctypes

    hook = _ntff_profile_via_ctypes("/opt/axon/libaxon_pjrt.so")
    mod = types.ModuleType("antenv.axon_hooks")
    store = {"h": hook}
    mod.get_axon_ntff_profile_hook = lambda: store["h"]
    mod.set_axon_ntff_profile_hook = lambda h: store.__setitem__("h", h)
    import antenv

    antenv.axon_hooks = mod
    sys.modules["antenv.axon_hooks"] = mod


def kernel(x, bn_weight, bn_bias, bn_mean, bn_var, conv_weight, _trace=False):
    if _trace:
        _install_ntff_hook()
    x = np.asarray(x, dtype=np.float32)
    s, t, wt = _prep_host(
        np.asarray(bn_weight, dtype=np.float32),
        np.asarray(bn_bias, dtype=np.float32),
        np.asarray(bn_mean, dtype=np.float32),
        np.asarray(bn_var, dtype=np.float32),
        np.asarray(conv_weight, dtype=np.float32),
    )
    in_maps = [
        {"x": np.ascontiguousarray(x[c * B_PC : (c + 1) * B_PC]), "s": s, "t": t, "wt": wt}
        for c in range(N_CORES)
    ]
    nc = _get_nc()
    res = run_bass_kernel_spmd(
        nc, in_maps, core_ids=list(range(N_CORES)), trace=_trace
    )
    out = np.concatenate([res.results[c]["out"] for c in range(N_CORES)], axis=0)
    if _trace:
        return out, res
    return out



# revision 3
# speedup vs baseline: 1.0998x; 1.0998x over previous
"""Fused BN(inference)+ReLU -> 1x1 conv (512->256) -> 2x2 avgpool on 8 TRN2 cores.

Full inputs in, full output out. Data-parallel over batch (16 -> 2 per core),
BN params + conv weights replicated.

Math folding (host side, tiny):
  s = bn_weight / sqrt(bn_var + eps)            [512]
  t = bn_bias - bn_mean * s                     [512]
  y = relu(s * x + t)                           (one ACT op per channel tile)
  avgpool2x2(W @ y) == (0.25 * W) @ sumpool2x2(y)   (pool before matmul: 4x
                                                     fewer matmul FLOPs)
  wt = 0.25 * W.T                               [512, 256] (lhsT layout)

Engine plan (per core, HBM stream is the roofline at ~14.5 MB / ~358 GB/s):
  sync  ring: the x stream ONLY (8 chunk DMAs, first/last split in half so
              the ACT head starts early and the tail drains fast). Nothing
              else ever queues here, so the stream never stalls on compute.
  scalar ring: params first (st is tiny and unblocks the ACT table load +
              first RELU; wt follows), then one RELU per chunk, then the
              out DMAs placed late enough in program order that their sem
              waits are already satisfied when the sequencer reaches them.
  DVE:        2x2 sum-pool (H-pairs then W-pairs) + all PSUM->SBUF copies.
  PE:         fp32 matmuls into 4 PSUM banks per batch.
"""

import copy as _copy

import numpy as np

import bass_rust
import concourse.bass as bass
import concourse.mybir as mybir
import concourse.tile as tile_mod
from concourse.bass_utils import run_bass_kernel_spmd

EPS = 1e-5

B, C_IN, C_OUT, H, W = 16, 512, 256, 56, 56
N_CORES = 8
B_PC = B // N_CORES          # batches per core
HW = H * W                   # 3136
HWP = (H // 2) * (W // 2)    # 784 pooled spatial
K_TILES = C_IN // 128        # 4
M_TILES = C_OUT // 128       # 2
N_CHUNK = HWP // 2           # 392 (fits one PSUM bank)

_DT = mybir.dt.float32


# This walrus build enforces per-instruction sync-wait caps that Tile's
# add_semaphores pass does not respect: CTRL-type instructions (Drain, NoOp)
# take no sem-ge waits at all, EventSemaphore takes at most 2, and every
# other instruction takes at most 1. Post-pass: hoist excess waits onto
# EventSemaphore carrier instructions inserted just before the owning
# instruction on the same engine (same blocking semantics - the carrier
# blocks the engine's sequencer until its waits pass).
_CTRL_OPS = ("InstDrain", "InstNoOp")


def _hoist_excess_waits(nc):
    ev_counter = [0]

    def make_carrier(engine, waits):
        ev_counter[0] += 1
        return mybir.InstEventSemaphore(
            name=f"EVHOIST-{ev_counter[0]}",
            engine=engine,
            ins=[],
            outs=[],
            sync_info=bass_rust.SyncInfo(on_wait=waits, on_update=[]),
        )

    new_module = _copy.replace(nc.m, functions=[])
    for function in nc.m.functions:
        new_function = _copy.replace(function, blocks=[])
        new_function.set_allocations_from_list(function.allocations)
        for block in function.blocks:
            new_insts = []
            for ins in block.instructions:
                si = ins.sync_info
                waits = list(si.on_wait) if si is not None else []
                opname = type(ins).__name__
                if opname in _CTRL_OPS:
                    keep = [w for w in waits if w.wait_mode != "sem-ge-imm"]
                    excess = [w for w in waits if w.wait_mode == "sem-ge-imm"]
                else:
                    limit = 2 if opname == "InstEventSemaphore" else 1
                    keep, excess = waits[:limit], waits[limit:]
                if excess:
                    for i in range(0, len(excess), 2):
                        new_insts.append(make_carrier(ins.engine, excess[i : i + 2]))
                    si.on_wait = keep
                new_insts.append(ins)
            new_function.blocks.append(_copy.replace(block, instructions=new_insts))
        new_module.functions.append(new_function)
    nc.m = new_module


def build_bass():
    nc = bass.Bass()

    # Params come pre-transposed from the host into partition-major layouts so
    # their DMAs are fully contiguous. st packs s and t ([128, 2K], 4KB) so
    # one tiny DMA unblocks both the ACT table warm-up and the first RELU.
    x_d = nc.dram_tensor("x", [B_PC, C_IN, H, W], _DT, kind="ExternalInput")
    st_d = nc.dram_tensor("st", [128, 2 * K_TILES], _DT, kind="ExternalInput")
    wt_d = nc.dram_tensor(
        "wt", [128, K_TILES * C_OUT], _DT, kind="ExternalInput"
    )
    out_d = nc.dram_tensor(
        "out", [B_PC, C_OUT, H // 2, W // 2], _DT, kind="ExternalOutput"
    )

    with tile_mod.TileContext(nc) as tc:
        with (
            tc.tile_pool(name="const", bufs=1) as cpool,
            tc.tile_pool(name="xs", bufs=6) as xpool,
            tc.tile_pool(name="ys", bufs=5) as ypool,
            tc.tile_pool(name="us", bufs=4) as upool,
            tc.tile_pool(name="ps", bufs=3) as ppool,
            tc.tile_pool(name="os", bufs=4) as opool,
            tc.tile_pool(name="psum", bufs=8, space="PSUM") as pspool,
        ):
            # Params on the SCALAR HWDGE ring: they must not queue behind the
            # 1.6MB x chunks on sync (that held the first RELU + ACT table
            # load hostage for ~8us). st (4KB) lands in <1us.
            st_sb = cpool.tile([128, 2 * K_TILES], _DT)
            nc.scalar.dma_start(out=st_sb[:], in_=st_d[:])
            wt_sb = cpool.tile([128, K_TILES * C_OUT], _DT)
            nc.scalar.dma_start(out=wt_sb[:], in_=wt_d[:])
            # Trigger the lazy ACT Relu table load now, off the critical path
            warm = cpool.tile([1, 1], _DT)
            nc.scalar.activation(
                warm[:], st_sb[0:1, 0:1], mybir.ActivationFunctionType.Relu
            )

            # x stream: sync ring only. First and last chunks split in half
            # (28 rows) so the pipeline head starts early / tail drains fast.
            x_tiles = {}
            pieces = []
            for b in range(B_PC):
                for k in range(K_TILES):
                    edge = (b == 0 and k == 0) or (
                        b == B_PC - 1 and k == K_TILES - 1
                    )
                    if edge:
                        pieces += [(b, k, 0, 28), (b, k, 28, 28)]
                    else:
                        pieces += [(b, k, 0, H)]
            for b, k, row0, nrows in pieces:
                x_t = xpool.tile(
                    [128, nrows * W], _DT, tag="x", name=f"x_{b}_{k}_{row0}"
                )
                nc.sync.dma_start(
                    out=x_t[:],
                    in_=x_d[
                        b, k * 128 : (k + 1) * 128, row0 : row0 + nrows
                    ].rearrange("ch h w -> ch (h w)"),
                )
                x_tiles[(b, k, row0)] = x_t

            def wt_ap(k, m):
                off = k * C_OUT + m * 128
                return wt_sb[:, off : off + 128]

            psums_by_batch = [{} for _ in range(B_PC)]
            o_tiles = {}

            def emit_compute(b, k, row0, nrows):
                """BN+ReLU -> 2x2 sum-pool -> matmul into psum pieces."""
                psums = psums_by_batch[b]
                first_k = k == 0
                last_k = k == K_TILES - 1
                hc = nrows * W
                x_t = x_tiles[(b, k, row0)]
                c = row0 // 14
                y_t = ypool.tile([128, hc], _DT, tag="y", name=f"y_{b}_{k}_{c}")
                nc.scalar.activation(
                    y_t[:],
                    x_t[:],
                    mybir.ActivationFunctionType.Relu,
                    bias=st_sb[:, K_TILES + k : K_TILES + k + 1],
                    scale=st_sb[:, k : k + 1],
                )
                # H-pairs first: operands are contiguous 56-elem runs
                u_t = upool.tile(
                    [128, hc // 2], _DT, tag="u", name=f"u_{b}_{k}_{c}"
                )
                yv = y_t[:].rearrange("p (h two w) -> p h two w", two=2, w=W)
                nc.vector.tensor_add(u_t[:], yv[:, :, 0, :], yv[:, :, 1, :])
                # then W-pairs
                p_t = ppool.tile(
                    [128, hc // 4], _DT, tag="p", name=f"p_{b}_{k}_{c}"
                )
                uv = u_t[:].rearrange("p (a two) -> p a two", two=2)
                nc.vector.tensor_add(p_t[:], uv[:, :, 0], uv[:, :, 1])
                # map this chunk's pooled columns onto psum n-chunk pieces.
                # PSUM first-write semantics: the matmul covering an n-chunk's
                # column 0 at k==0 carries start=True; later same-k pieces
                # land as overwrites on the cleared has_written bits.
                pooled0 = (row0 // 2) * (W // 2)  # global pooled col offset
                pooled_w = (nrows // 2) * (W // 2)
                for m in range(M_TILES):
                    off = 0
                    while off < pooled_w:
                        g = pooled0 + off  # global pooled col
                        n = g // N_CHUNK
                        col = g % N_CHUNK
                        width = min(N_CHUNK - col, pooled_w - off)
                        if first_k and (m, n) not in psums:
                            psums[(m, n)] = pspool.tile(
                                [128, N_CHUNK],
                                _DT,
                                tag="psum",
                                name=f"psum_{b}_{m}_{n}",
                            )
                        nc.tensor.matmul(
                            psums[(m, n)][:, col : col + width],
                            wt_ap(k, m),
                            p_t[:, off : off + width],
                            start=(first_k and col == 0),
                            stop=(last_k and col + width == N_CHUNK),
                            skip_group_check=True,
                        )
                        off += width

            def emit_psum_drain(b):
                # PSUM -> SBUF on DVE (DMA can't read PSUM). Both n-halves of
                # a (b, m) land in ONE [128, 784] tile -> one out DMA each.
                psums = psums_by_batch[b]
                for n in range(2):
                    for m in range(M_TILES):
                        if (b, m) not in o_tiles:
                            o_tiles[(b, m)] = opool.tile(
                                [128, HWP], _DT, tag="o", name=f"o_{b}_{m}"
                            )
                        nc.vector.tensor_copy(
                            o_tiles[(b, m)][:, n * N_CHUNK : (n + 1) * N_CHUNK],
                            psums[(m, n)][:],
                        )

            out_v = out_d[:].rearrange("bb o h w -> bb o (h w)")

            def emit_out_dma(b):
                for m in range(M_TILES):
                    nc.scalar.dma_start(
                        out=out_v[b, m * 128 : (m + 1) * 128, :],
                        in_=o_tiles[(b, m)][:],
                    )

            # Program order: all compute for b0, then b0's psum drain; b0's
            # out DMAs go on the scalar ring AFTER two of b1's RELUs so the
            # sequencer reaches them when their waits are long satisfied.
            for b, k, row0, nrows in pieces:
                if b == 1 and k == 2 and row0 == 0:
                    emit_out_dma(0)
                emit_compute(b, k, row0, nrows)
                if k == K_TILES - 1 and row0 + nrows == H:
                    emit_psum_drain(b)
            emit_out_dma(1)
    _hoist_excess_waits(nc)
    return nc


_NC_CACHE = None


def _get_nc():
    global _NC_CACHE
    if _NC_CACHE is None:
        _NC_CACHE = build_bass()
    return _NC_CACHE


def _prep_host(bn_weight, bn_bias, bn_mean, bn_var, conv_weight):
    s = (bn_weight / np.sqrt(bn_var + EPS)).astype(np.float32)
    t = (bn_bias - bn_mean * s).astype(np.float32)
    wt = (0.25 * conv_weight.T).astype(np.float32)  # [C_IN, C_OUT]
    # partition-major layouts: [128, 2K] for s|t, [128, K*C_OUT] for wt
    s2 = s.reshape(K_TILES, 128).T
    t2 = t.reshape(K_TILES, 128).T
    st = np.ascontiguousarray(np.concatenate([s2, t2], axis=1))
    wt2 = np.ascontiguousarray(
        wt.reshape(K_TILES, 128, C_OUT).transpose(1, 0, 2).reshape(128, -1)
    )
    return st, wt2


def _install_ntff_hook():
    # The agent image's antenv lacks axon_hooks; synthesize it from the boot
    # shim's ctypes factory so trace=True captures NTFF profiles.
    import sys
    import types

    try:
        import antenv.axon_hooks  # noqa: F401

        return
    except ImportError:
        pass
    from trn_agent_boot.trn_boot import _ntff_profile_via_ctypes

    hook = _ntff_profile_via_ctypes("/opt/axon/libaxon_pjrt.so")
    mod = types.ModuleType("antenv.axon_hooks")
    store = {"h": hook}
    mod.get_axon_ntff_profile_hook = lambda: store["h"]
    mod.set_axon_ntff_profile_hook = lambda h: store.__setitem__("h", h)
    import antenv

    antenv.axon_hooks = mod
    sys.modules["antenv.axon_hooks"] = mod


def kernel(x, bn_weight, bn_bias, bn_mean, bn_var, conv_weight, _trace=False):
    if _trace:
        _install_ntff_hook()
    x = np.asarray(x, dtype=np.float32)
    st, wt = _prep_host(
        np.asarray(bn_weight, dtype=np.float32),
        np.asarray(bn_bias, dtype=np.float32),
        np.asarray(bn_mean, dtype=np.float32),
        np.asarray(bn_var, dtype=np.float32),
        np.asarray(conv_weight, dtype=np.float32),
    )
    in_maps = [
        {"x": np.ascontiguousarray(x[c * B_PC : (c + 1) * B_PC]), "st": st, "wt": wt}
        for c in range(N_CORES)
    ]
    nc = _get_nc()
    res = run_bass_kernel_spmd(
        nc, in_maps, core_ids=list(range(N_CORES)), trace=_trace
    )
    out = np.concatenate([res.results[c]["out"] for c in range(N_CORES)], axis=0)
    if _trace:
        return out, res
    return out
